# Initial kernel scaffold
#
"""GAU (gated attention unit) Trainium2 kernel.

Data-parallel over batch: 32 batches -> 8 NeuronCores x 4 batches.
All weights replicated; no collectives.

Per-batch dataflow (L=512 tokens, HID=768, E=1536, S=128):
  1. DMA x[b] token-major [128tok x 4tile, 768], LayerNorm (fp32 stats).
  2. PE-transpose xn -> xnT feature-major [768, 512] (bf16).
  3. uv projection (bf16 matmuls, fp32 PSUM):
       uT  [e,n] feature-major (lhsT=uv_W tiles, rhs=xnT)   + silu(+bias) on ACT
       v   [n,e] token-major   (lhsT=xnT tiles, rhs=uv_W)   + silu on ACT
       baseT [s,n] feature-major                            + silu(+bias) on ACT
  4. q/k: per-partition affine (gamma_qk/beta_qk; 1/L folded into q's gamma),
     RoPE = x*cos + rotate_half(x)*sin, rotate_half via signed permutation
     matmul (float32r).  -> qT,kT bf16 [128, 512].
  5. scoresT[m,n] = k[m]. q[n]  (single matmul per 128-key tile),
     + Toeplitz bias (host-expanded), relu^2 -> bf16.
  6. attnvT[e,n] = sum_m v[m,e] * scoresT[m,n]  (lhsT=v token-major tiles),
     gate with uT on DVE -> gatedT bf16 feature-major.
  7. o-projection token-major (lhsT=gatedT tiles, rhs=o_W), + residual x (fp32),
     DMA out.

Performance (8 cores, 4 batches each): ~180 us/core HW exec, fp8e4m3 DoubleRow
matmuls at the 156 TF/s roofline cadence (215 ns per K=256,N=512 matmul),
PE ~80%% busy; startup hidden by a PE warm-up burst (HAM clock-gate) and
software-pipelined cross-batch emission (batch b+1 layernorm runs during
batch b o-projection).  Scale-relative absmax error ~1.4e-4 (bf16-only
fallback: set USE_FP8=False, ~288 us at ~9e-6).
"""

import sys
from contextlib import ExitStack

if "/opt/trn_rl_repo" not in sys.path:
    sys.path.insert(0, "/opt/trn_rl_repo")

import numpy as np
import ml_dtypes

import concourse.tile as tile
from concourse import mybir, bacc
from concourse import bass_utils
from concourse.masks import make_identity

N_CORES = 8
B, L, HID, E, S = 32, 512, 768, 1536, 128
NB = B // N_CORES            # batches per core
EPS = 1e-5
P = 128
KT = HID // P                # 6 k-tiles over hid
ET = E // P                  # 12 e-tiles
TT = L // P                  # 4 token tiles
F32 = mybir.dt.float32
BF16 = mybir.dt.bfloat16
F8 = mybir.dt.float8e4
HALF = S // 2
LAM = 256.0          # fp8 score scaling: keeps relu^2 scores out of fp8 subnormals
USE_FP8 = True       # fp8e4m3 + DoubleRow for projection/attention matmuls


def _build_program(has_uvb: bool, has_ob: bool, nb: int = NB, fp8: bool = False):
    nc = bacc.Bacc("TRN2", target_bir_lowering=False, debug=False, num_devices=1)

    x_d = nc.dram_tensor("x", [nb, L, HID], F32, kind="ExternalInput").ap()
    WDT = F8 if fp8 else BF16
    uvw_d = nc.dram_tensor("uvw", [HID, 2 * E + S], WDT, kind="ExternalInput").ap()
    ow_d = nc.dram_tensor("ow", [E, HID], WDT, kind="ExternalInput").ap()
    bias_d = nc.dram_tensor("biasT", [TT, P, L], BF16, kind="ExternalInput").ap()
    cos_d = nc.dram_tensor("cosf", [P, L], F32, kind="ExternalInput").ap()
    sin_d = nc.dram_tensor("sinf", [P, L], F32, kind="ExternalInput").ap()
    prope_d = nc.dram_tensor("prope", [P, P], BF16, kind="ExternalInput").ap()
    qkaff_d = nc.dram_tensor("qkaff", [P, 4], F32, kind="ExternalInput").ap()
    if has_uvb:
        bu_d = nc.dram_tensor("bu", [P, ET + 1], F32, kind="ExternalInput").ap()
    if has_uvb:
        bv_d = nc.dram_tensor("bv", [1, E], BF16, kind="ExternalInput").ap()
    if has_ob:
        ob_d = nc.dram_tensor("ob", [1, HID], BF16, kind="ExternalInput").ap()
    y_d = nc.dram_tensor("y", [nb, L, HID], F32, kind="ExternalOutput").ap()

    with tile.TileContext(nc) as tc, ExitStack() as ctx:
        consts = ctx.enter_context(tc.tile_pool(name="consts", bufs=1))
        xpool = ctx.enter_context(tc.tile_pool(name="xpool", bufs=2))
        xnpool = ctx.enter_context(tc.tile_pool(name="xnpool", bufs=2))
        xntpool = ctx.enter_context(tc.tile_pool(name="xntpool", bufs=2))
        upool = ctx.enter_context(tc.tile_pool(name="upool", bufs=2))
        vpool = ctx.enter_context(tc.tile_pool(name="vpool", bufs=2))
        work = ctx.enter_context(tc.tile_pool(name="work", bufs=2))
        statp = ctx.enter_context(tc.tile_pool(name="statp", bufs=2))
        scp = ctx.enter_context(tc.tile_pool(name="scp", bufs=2))
        gp = ctx.enter_context(tc.tile_pool(name="gp", bufs=2))
        yp = ctx.enter_context(tc.tile_pool(name="yp", bufs=4))

        ps_t = ctx.enter_context(tc.tile_pool(name="ps_t", bufs=2, space="PSUM"))
        ps_mm = ctx.enter_context(tc.tile_pool(name="ps_mm", bufs=4, space="PSUM"))
        ps_y = ctx.enter_context(tc.tile_pool(name="ps_y", bufs=2, space="PSUM"))

        # ---- small constants first (unblock front(0) on DVE/ACT quickly) ----
        epst = consts.tile([P, 1], F32)
        nc.vector.memset(epst[:], EPS)
        ident = consts.tile([P, P], BF16)
        make_identity(nc, ident[:])
        # HAM warm-up: PE sits idle ~11us at startup waiting for x/LN; keep it
        # busy with throwaway matmuls so the clock gate is at 8/8 (2.4GHz)
        # when the real stream starts (saves the ~3.4us ramp on real work).
        wps = ps_mm.tile([P, P], F32, tag="ps_mm")
        for _ in range(110):
            nc.tensor.matmul(wps[:], ident[:], ident[:], start=True, stop=True)
        # prime engines during the initial x DMA: load ACT tables (sqrt then
        # silu, matching first-use order) and DVE's internal state so the
        # batch-0 layernorm chain doesn't pay table-load latency
        prm = consts.tile([P, 6], F32)
        nc.vector.bn_stats(out=prm[:], in_=epst[:])
        prs = consts.tile([P, 1], F32)
        nc.scalar.activation(out=prs[:], in_=epst[:],
                             func=mybir.ActivationFunctionType.Sqrt)
        nc.scalar.activation(out=prs[:], in_=prs[:],
                             func=mybir.ActivationFunctionType.Silu)
        qkaff = consts.tile([P, 4], F32)
        nc.sync.dma_start(qkaff[:], qkaff_d)
        cosf = consts.tile([P, L], F32)
        nc.sync.dma_start(cosf[:], cos_d)
        sinf = consts.tile([P, L], F32)
        nc.sync.dma_start(sinf[:], sin_d)
        prope = consts.tile([P, P], BF16)
        nc.sync.dma_start(prope[:], prope_d)
        if has_uvb:
            bu = consts.tile([P, ET + 1], F32)
            nc.sync.dma_start(bu[:], bu_d)
            bv = consts.tile([1, E], BF16)
            nc.sync.dma_start(bv[:], bv_d)
        if has_ob:
            ob = consts.tile([1, HID], BF16)
            nc.sync.dma_start(ob[:], ob_d)
        if has_uvb or has_ob:
            ones1 = consts.tile([1, P], BF16)
            nc.vector.memset(ones1[:], 1.0)

        g0 = qkaff[:, 0:1]
        b0 = qkaff[:, 1:2]
        g1 = qkaff[:, 2:3]
        b1 = qkaff[:, 3:4]
        SILU = mybir.ActivationFunctionType.Silu

        # ---- per-batch stage emitters; state passed via dicts ----
        st = [dict() for _ in range(nb)]

        def front(b):
            """DMA x[b] + layernorm (token-major). Emitted one batch ahead."""
            d = st[b]
            x_tok = xpool.tile([P, TT, HID], F32, tag="x_tok", name=f"x_tok{b}")
            x_r = x_d[b].rearrange("(t p) h -> p t h", p=P)
            for t in range(TT):
                nc.sync.dma_start(x_tok[:, t, :], x_r[:, t, :])
            xn = xnpool.tile([P, TT, HID], BF16, tag="xn", name=f"xn{b}")
            mvs = statp.tile([P, TT, 2], F32, tag="mvs", name=f"mvs{b}")
            for t in range(TT):
                xin = x_tok[:, t, :].rearrange("p (s c) -> p s c", c=256)
                stats = statp.tile([P, 3, 6], F32, tag="stats")
                for sgi in range(3):
                    nc.vector.bn_stats(out=stats[:, sgi, :], in_=xin[:, sgi, :])
                nc.vector.bn_aggr(out=mvs[:, t, :], in_=stats[:])
                if b == 0:
                    # batch 0 is on the critical startup path: finish each
                    # token tile immediately instead of batching the sqrt
                    stdp = statp.tile([P, 1], F32, tag="stdp0")
                    nc.scalar.activation(
                        out=stdp[:], in_=mvs[:, t, 1:2],
                        func=mybir.ActivationFunctionType.Sqrt,
                        bias=epst[:], scale=1.0,
                    )
                    rstd = statp.tile([P, 1], F32, tag="rstd0")
                    nc.vector.reciprocal(out=rstd[:], in_=stdp[:])
                    nc.vector.tensor_scalar(
                        out=xn[:, t, :], in0=x_tok[:, t, :],
                        scalar1=mvs[:, t, 0:1], scalar2=rstd[:],
                        op0=mybir.AluOpType.subtract, op1=mybir.AluOpType.mult,
                    )
            if b > 0:
                stdp = statp.tile([P, TT], F32, tag="stdp", name=f"stdp{b}")
                nc.scalar.activation(
                    out=stdp[:], in_=mvs[:, :, 1],
                    func=mybir.ActivationFunctionType.Sqrt,
                    bias=epst[:], scale=1.0,
                )
                rstd = statp.tile([P, TT], F32, tag="rstd", name=f"rstd{b}")
                nc.vector.reciprocal(out=rstd[:], in_=stdp[:])
                for t in range(TT):
                    nc.vector.tensor_scalar(
                        out=xn[:, t, :], in0=x_tok[:, t, :],
                        scalar1=mvs[:, t, 0:1], scalar2=rstd[:, t:t + 1],
                        op0=mybir.AluOpType.subtract, op1=mybir.AluOpType.mult,
                    )
            d["x_tok"], d["xn"] = x_tok, xn

        def transposes(b):
            d = st[b]
            xn = d["xn"]
            xnT = xntpool.tile([P, KT, L], WDT, tag="xnT", name=f"xnT{b}")
            if b == 0:
                # batch 0: group by token tile so transposes start as soon as
                # the first LN tile lands (startup critical path)
                for t in range(TT):
                    for ks in range(0, KT, 3):
                        pt3 = ps_t.tile([P, 3, P], BF16, tag="pt4")
                        for k in range(ks, ks + 3):
                            nc.tensor.transpose(
                                pt3[:, k - ks, :], xn[:, t, k * P:(k + 1) * P], ident[:])
                        nc.vector.tensor_copy(
                            out=xnT[:, ks:ks + 3, t * P:(t + 1) * P], in_=pt3[:])
            else:
                for k in range(KT):
                    pt4 = ps_t.tile([P, TT, P], BF16, tag="pt4")
                    for t in range(TT):
                        nc.tensor.transpose(pt4[:, t, :], xn[:, t, k * P:(k + 1) * P], ident[:])
                    nc.vector.tensor_copy(
                        out=xnT[:, k, :], in_=pt4.rearrange("p t q -> p (t q)"))
            d["xnT"] = xnT

        def base_group(b):
            d = st[b]
            xnT = d["xnT"]
            ps_b = ps_mm.tile([P, L], F32, tag="ps_mm")
            if fp8:
                for k in range(0, KT, 2):
                    nc.tensor.matmul(
                        ps_b[:], uvw[:, k:k + 2, 2 * E: 2 * E + S], xnT[:, k:k + 2, :],
                        start=(k == 0), stop=(k == KT - 2),
                        perf_mode=mybir.MatmulPerfMode.DoubleRow,
                    )
            else:
                for k in range(KT):
                    nc.tensor.matmul(
                        ps_b[:], uvw[:, k, 2 * E: 2 * E + S], xnT[:, k, :],
                        start=(k == 0), stop=(k == KT - 1),
                    )
            with tc.high_priority(offset=600):
                baseT = work.tile([P, L], F32, tag="baseT", name=f"baseT{b}")
                if has_uvb:
                    nc.scalar.activation(out=baseT[:], in_=ps_b[:], func=SILU,
                                         bias=bu[:, ET:ET + 1], scale=1.0)
                else:
                    nc.scalar.activation(out=baseT[:], in_=ps_b[:], func=SILU)
                # q/k affine on DVE right away (rope matmuls come later)
                for which, gg, bb in (("q", g0, b0), ("k", g1, b1)):
                    pre = work.tile([P, L], BF16, tag=f"{which}pre", name=f"{which}pre{b}")
                    nc.vector.tensor_scalar(
                        out=pre[:], in0=baseT[:], scalar1=gg, scalar2=bb,
                        op0=mybir.AluOpType.mult, op1=mybir.AluOpType.add,
                    )
                    d[f"{which}pre"] = pre

        def u_groups(b, es):
            d = st[b]
            xnT = d["xnT"]
            if "uT" not in d:
                d["uT"] = upool.tile([P, ET, L], BF16, tag="uT", name=f"uT{b}")
            uT = d["uT"]
            for e in es:
                ps_u = ps_mm.tile([P, L], F32, tag="ps_mm")
                if fp8:
                    for k in range(0, KT, 2):
                        nc.tensor.matmul(
                            ps_u[:], uvw[:, k:k + 2, e * P:(e + 1) * P], xnT[:, k:k + 2, :],
                            start=(k == 0), stop=(k == KT - 2),
                            perf_mode=mybir.MatmulPerfMode.DoubleRow,
                        )
                else:
                    for k in range(KT):
                        nc.tensor.matmul(
                            ps_u[:], uvw[:, k, e * P:(e + 1) * P], xnT[:, k, :],
                            start=(k == 0), stop=(k == KT - 1),
                        )
                if has_uvb:
                    nc.scalar.activation(out=uT[:, e, :], in_=ps_u[:], func=SILU,
                                         bias=bu[:, e:e + 1], scale=1.0)
                else:
                    nc.scalar.activation(out=uT[:, e, :], in_=ps_u[:], func=SILU)

        def rope_mms(b):
            """rotate-half matmuls + combines -> qT/kT bf16."""
            d = st[b]
            for which in ("q", "k"):
                pre = d[f"{which}pre"]
                ps_r = ps_mm.tile([P, L], F32, tag="ps_mm")
                nc.tensor.matmul(ps_r[:], prope[:], pre[:], start=True, stop=True)
                with tc.high_priority(offset=600):
                    rt = work.tile([P, L], F32, tag="ropetmp")
                    nc.vector.tensor_tensor(rt[:], ps_r[:], sinf[:], mybir.AluOpType.mult)
                    ct = work.tile([P, L], F32, tag="ropecos", name=f"rc_{which}{b}")
                    nc.gpsimd.tensor_tensor(ct[:], pre[:], cosf[:], mybir.AluOpType.mult)
                    qt = work.tile([P, L], BF16, tag=f"{which}T", name=f"{which}T{b}")
                    nc.vector.tensor_tensor(qt[:], ct[:], rt[:], mybir.AluOpType.add)
                d[which] = qt

        def score_groups(b, mts=range(TT)):
            d = st[b]
            if "scoresT" not in d:
                d["scoresT"] = scp.tile([P, TT, L], WDT, tag="scoresT", name=f"scoresT{b}")
            scoresT = d["scoresT"]
            for mt in mts:
                ps_s = ps_mm.tile([P, L], F32, tag="ps_mm")
                nc.tensor.matmul(
                    ps_s[:], d["k"][:, mt * P:(mt + 1) * P], d["q"][:],
                    start=True, stop=True,
                )
                with tc.high_priority(offset=600):
                    stmp = work.tile([P, L], F32, tag="stmp", bufs=4)
                    nc.vector.tensor_tensor(stmp[:], ps_s[:], biasT[:, mt, :], mybir.AluOpType.add)
                    srelu = work.tile([P, L], BF16, tag="srelu", bufs=4)
                    nc.scalar.activation(out=srelu[:], in_=stmp[:],
                                         func=mybir.ActivationFunctionType.Relu)
                    nc.vector.tensor_tensor(
                        scoresT[:, mt, :], stmp[:], srelu[:], mybir.AluOpType.mult)

        def v_groups(b, ts=range(TT)):
            d = st[b]
            xnT = d["xnT"]
            if "v_sb" not in d:
                d["v_sb"] = vpool.tile([P, TT, 3, 512], WDT, tag="v_sb", name=f"v_sb{b}")
            v_sb = d["v_sb"]
            for t in ts:
                for c in range(3):
                    ps_v = ps_mm.tile([P, 512], F32, tag="ps_mm")
                    if fp8:
                        for k in range(0, KT, 2):
                            nc.tensor.matmul(
                                ps_v[:], xnT[:, k:k + 2, t * P:(t + 1) * P],
                                uvw[:, k:k + 2, E + c * 512: E + (c + 1) * 512],
                                start=(k == 0), stop=(k == KT - 2 and not has_uvb),
                                perf_mode=mybir.MatmulPerfMode.DoubleRow,
                            )
                    else:
                        for k in range(KT):
                            nc.tensor.matmul(
                                ps_v[:], xnT[:, k, t * P:(t + 1) * P],
                                uvw[:, k, E + c * 512: E + (c + 1) * 512],
                                start=(k == 0), stop=(k == KT - 1 and not has_uvb),
                            )
                    if has_uvb:
                        nc.tensor.matmul(
                            ps_v[:], ones1[:],
                            bv[:, c * 512:(c + 1) * 512],
                            start=False, stop=True, skip_group_check=True,
                        )
                    nc.scalar.activation(out=v_sb[:, t, c, :], in_=ps_v[:], func=SILU)

        def attnv_groups(b):
            d = st[b]
            gatedT = gp.tile([P, ET, L], WDT, tag="gatedT", name=f"gatedT{b}")
            for e in range(ET):
                c, el = divmod(e, 4)
                ps_a = ps_mm.tile([P, L], F32, tag="ps_mm")
                if fp8:
                    for mt in range(0, TT, 2):
                        nc.tensor.matmul(
                            ps_a[:], d["v_sb"][:, mt:mt + 2, c, el * P:(el + 1) * P],
                            d["scoresT"][:, mt:mt + 2, :],
                            start=(mt == 0), stop=(mt == TT - 2),
                            perf_mode=mybir.MatmulPerfMode.DoubleRow,
                        )
                else:
                    for mt in range(TT):
                        nc.tensor.matmul(
                            ps_a[:], d["v_sb"][:, mt, c, el * P:(el + 1) * P],
                            d["scoresT"][:, mt, :],
                            start=(mt == 0), stop=(mt == TT - 1),
                        )
                nc.vector.tensor_tensor(
                    gatedT[:, e, :], ps_a[:], d["uT"][:, e, :], mybir.AluOpType.mult)
            d["gatedT"] = gatedT

        def oproj(b):
            d = st[b]
            for t in range(TT):
                y_tok = yp.tile([P, HID], F32, tag="y_tok")
                for c in range(2):
                    ps_o = ps_y.tile([P, HID // 2], F32, tag="ps_y")
                    if fp8:
                        for e in range(0, ET, 2):
                            nc.tensor.matmul(
                                ps_o[:], d["gatedT"][:, e:e + 2, t * P:(t + 1) * P],
                                ow[:, e:e + 2, c * (HID // 2):(c + 1) * (HID // 2)],
                                start=(e == 0), stop=(e == ET - 2 and not has_ob),
                                perf_mode=mybir.MatmulPerfMode.DoubleRow,
                            )
                    else:
                        for e in range(ET):
                            nc.tensor.matmul(
                                ps_o[:], d["gatedT"][:, e, t * P:(t + 1) * P],
                                ow[:, e, c * (HID // 2):(c + 1) * (HID // 2)],
                                start=(e == 0), stop=(e == ET - 1 and not has_ob),
                            )
                    if has_ob:
                        nc.tensor.matmul(
                            ps_o[:], ones1[:],
                            ob[:, c * (HID // 2):(c + 1) * (HID // 2)],
                            start=False, stop=True, skip_group_check=True,
                        )
                    if fp8:
                        y1 = work.tile([P, HID // 2], F32, tag="y1")
                        nc.scalar.activation(
                            out=y1[:], in_=ps_o[:],
                            func=mybir.ActivationFunctionType.Copy,
                            scale=1.0 / LAM,
                        )
                        nc.gpsimd.tensor_tensor(
                            y_tok[:, c * (HID // 2):(c + 1) * (HID // 2)],
                            y1[:], d["x_tok"][:, t, c * (HID // 2):(c + 1) * (HID // 2)],
                            mybir.AluOpType.add,
                        )
                    else:
                        nc.vector.tensor_tensor(
                            y_tok[:, c * (HID // 2):(c + 1) * (HID // 2)],
                            ps_o[:], d["x_tok"][:, t, c * (HID // 2):(c + 1) * (HID // 2)],
                            mybir.AluOpType.add,
                        )
                nc.sync.dma_start(y_d[b, t * P:(t + 1) * P, :], y_tok[:])
            st[b] = {}

        # ---- software-pipelined emission ----
        # front(0) first so x[0] DMA + LN start immediately; the big weight
        # DMAs are emitted after so they don't block batch 0's layernorm.
        front(0)
        # big weights after front(0): split across queues, ordered by first use
        uvw = consts.tile([P, KT, 2 * E + S], WDT)
        uvw_r = uvw_d.rearrange("(k p) f -> p k f", p=P)
        for k in range(KT):
            nc.sync.dma_start(uvw[:, k, :], uvw_r[:, k, :])
        biasT = consts.tile([P, TT, L], BF16)
        nc.sync.dma_start(biasT[:], bias_d.rearrange("t p n -> p t n"))
        ow = consts.tile([P, ET, HID], WDT)
        ow_r = ow_d.rearrange("(k p) f -> p k f", p=P)
        for k in range(ET):
            nc.sync.dma_start(ow[:, k, :], ow_r[:, k, :])

        # front(b+1) lands before oproj(b) so batch b+1's LN runs on DVE
        # while PE finishes batch b.
        for b in range(nb):
            transposes(b)
            base_group(b)
            u_groups(b, range(0, 6))
            rope_mms(b)
            u_groups(b, range(6, ET))
            score_groups(b)
            v_groups(b)
            attnv_groups(b)
            if b + 1 < nb:
                front(b + 1)
            oproj(b)

    nc.compile()
    return nc


def _host_prep(x, ln_gamma, ln_beta, uv_W, uv_b, gamma_qk, beta_qk, w_rel, o_W, o_b,
               fp8=False):
    """Host-side input preprocessing: fold LN affine into uv_W, 1/L into the q
    affine, expand the Toeplitz bias, build rope tables.

    fp8 mode: weights are cast to float8_e4m3 and the attention scores are
    scaled by LAM (sqrt(LAM) folded into the q affine and the Toeplitz bias;
    relu^2 turns that into LAM; the o-projection PSUM copy divides it out).
    fp8's min normal is 2^-6 -- unscaled relu^2 scores (~1e-4) would land in
    subnormals and quantize to garbage."""
    f32 = np.float32
    sq = f32(np.sqrt(LAM)) if fp8 else f32(1.0)
    uv_W = np.asarray(uv_W, f32)
    uv_b_eff = (np.asarray(ln_beta, f32) @ uv_W + np.asarray(uv_b, f32)).astype(f32)
    uv_W_eff = (np.asarray(ln_gamma, f32)[:, None] * uv_W).astype(f32)

    gamma_qk = np.asarray(gamma_qk, f32)
    beta_qk = np.asarray(beta_qk, f32)
    qkaff = np.stack(
        [gamma_qk[0] * sq / f32(L), beta_qk[0] * sq / f32(L), gamma_qk[1], beta_qk[1]],
        axis=1,
    ).astype(f32)                                           # [128, 4]

    # rope tables, feature-major: cos/sin[s, n] = cos/sin(n * invf[s % 64])
    inv_freq = np.power(f32(10000.0), -np.arange(HALF, dtype=f32) / f32(HALF))
    sinusoid = np.arange(L, dtype=f32)[None, :] * inv_freq[:, None]   # [64, 512]
    cosf = np.concatenate([np.cos(sinusoid), np.cos(sinusoid)], 0).astype(f32)
    sinf = np.concatenate([np.sin(sinusoid), np.sin(sinusoid)], 0).astype(f32)

    # signed rotate-half permutation, as lhsT: out[m,n] = sum_s lhsT[s,m] in[s,n]
    prope = np.zeros((S, S), f32)
    for m in range(HALF):
        prope[m + HALF, m] = -1.0
    for m in range(HALF, S):
        prope[m - HALF, m] = 1.0

    # Toeplitz bias, transposed orientation: biasT[mt, p, n] = w_rel[128*mt+p-n+511]
    w_rel = np.asarray(w_rel, f32)
    idx = (np.arange(L)[:, None] - np.arange(L)[None, :] + (L - 1))   # [m, n]
    biasT = (w_rel[idx].reshape(TT, P, L) * sq).astype(ml_dtypes.bfloat16)

    bu = np.stack(
        [uv_b_eff[e * P:(e + 1) * P] for e in range(ET)] + [uv_b_eff[2 * E: 2 * E + S]],
        axis=1,
    ).astype(f32)                                           # [128, 13]

    has_uvb = bool(np.any(uv_b_eff != 0))
    o_b = np.asarray(o_b, f32)
    has_ob = bool(np.any(o_b != 0))

    wnp = mybir.dt.np(F8) if fp8 else ml_dtypes.bfloat16
    shared = {
        "uvw": uv_W_eff.astype(wnp),
        "ow": np.asarray(o_W, f32).astype(wnp),
        "biasT": biasT,
        "cosf": cosf,
        "sinf": sinf,
        "prope": prope.astype(ml_dtypes.bfloat16),
        "qkaff": qkaff,
    }
    if has_uvb:
        shared["bu"] = bu
        shared["bv"] = uv_b_eff[E:2 * E].reshape(1, E).astype(ml_dtypes.bfloat16)
    if has_ob:
        shared["ob"] = o_b.reshape(1, HID).astype(ml_dtypes.bfloat16)
    return shared, has_uvb, has_ob


_prog_cache = {}


def run(inputs, trace=False, trace_kwargs=None, fp8=USE_FP8):
    x = np.asarray(inputs["x"], np.float32)
    shared, has_uvb, has_ob = _host_prep(**inputs, fp8=fp8)
    key = (has_uvb, has_ob, fp8)
    if key not in _prog_cache:
        _prog_cache[key] = _build_program(has_uvb, has_ob, fp8=fp8)
    nc = _prog_cache[key]
    in_maps = [
        {"x": np.ascontiguousarray(x[i * NB:(i + 1) * NB]), **shared}
        for i in range(N_CORES)
    ]
    kw = {}
    if trace:
        kw = dict(trace=True, trace_kwargs=trace_kwargs or {})
    try:
        res = bass_utils.run_bass_kernel_spmd(nc, in_maps, core_ids=list(range(N_CORES)), **kw)
    except Exception:
        import time as _time
        _time.sleep(10)
        res = bass_utils.run_bass_kernel_spmd(nc, in_maps, core_ids=list(range(N_CORES)), **kw)
    y = np.concatenate([res.results[i]["y"] for i in range(N_CORES)], axis=0)
    return y, res


def kernel(**inputs) -> np.ndarray:
    y, _ = run(inputs, trace=False)
    return y



# revision 4
# speedup vs baseline: 1.0469x; 1.0469x over previous
"""GAU (gated attention unit) Trainium2 kernel.

Data-parallel over batch: 32 batches -> 8 NeuronCores x 4 batches.
All weights replicated; no collectives.

Per-batch dataflow (L=512 tokens, HID=768, E=1536, S=128):
  1. DMA x[b] token-major bf16 [128tok x 4tile, 768], LayerNorm (fp32 stats,
     rstd via DVE fast-inverse-sqrt + 2 Newton steps -- keeps ACT on one
     activation table the whole run).
  2. PE-transpose xn -> xnT feature-major [768, 512] (fp8/bf16).
  3. uv projection (fp8 DoubleRow matmuls, fp32 PSUM):
       uT  [e,n] feature-major (lhsT=uv_W tiles, rhs=xnT)   + silu on ACT
       v   [n,e] token-major   (lhsT=xnT tiles, rhs=uv_W)   + silu on ACT
       baseT [s,n] feature-major                            + silu on ACT
  4. q/k: per-partition affine (gamma_qk/beta_qk; 1/L folded into q's gamma),
     RoPE = x*cos + rotate_half(x)*sin, rotate_half via signed permutation
     matmul into the transpose-pool PSUM (keeps the main matmul PSUM ring
     free of cross-engine recycle stalls).
  5. scoresT[m,n] = k[m]. q[n], + Toeplitz bias (host-expanded) on DVE,
     relu^2 fused as one scalar_tensor_tensor (max 0 then multiply).
  6. attnvT[e,n] = sum_m v[m,e] * scoresT[m,n], gate with uT split across
     DVE and GPSIMD.
  7. o-projection token-major; PSUM drained by one fused
     (psum * 1/LAM + x) scalar_tensor_tensor per half, DMA out per half.

Weight DMA is column-grouped in first-use order (base+u01, u2-5, u6-11,
v halves) so batch 0's projection starts ~7us in; ow and bias tables queue
behind.  fp8e4m3 DoubleRow matmuls run at the 156 TF/s roofline cadence
(215 ns per K=256,N=512 matmul).  Scale-relative absmax error ~2e-3
(bf16 x input + fp8 math; bf16-only fallback: USE_FP8=False).
"""

import sys
from contextlib import ExitStack

if "/opt/trn_rl_repo" not in sys.path:
    sys.path.insert(0, "/opt/trn_rl_repo")

import numpy as np
import ml_dtypes

import concourse.tile as tile
from concourse import mybir, bacc
from concourse import bass_utils
from concourse.masks import make_identity

N_CORES = 8
B, L, HID, E, S = 32, 512, 768, 1536, 128
NB = B // N_CORES            # batches per core
EPS = 1e-5
P = 128
KT = HID // P                # 6 k-tiles over hid
ET = E // P                  # 12 e-tiles
TT = L // P                  # 4 token tiles
F32 = mybir.dt.float32
U32 = mybir.dt.uint32
BF16 = mybir.dt.bfloat16
F8 = mybir.dt.float8e4
HALF = S // 2
LAM = 256.0          # fp8 score scaling: keeps relu^2 scores out of fp8 subnormals
USE_FP8 = True       # fp8e4m3 + DoubleRow for projection/attention matmuls
RSQRT_MAGIC = 0x5F3759DF


def _build_program(has_uvb: bool, has_ob: bool, nb: int = NB, fp8: bool = False):
    nc = bacc.Bacc("TRN2", target_bir_lowering=False, debug=False, num_devices=1)

    x_d = nc.dram_tensor("x", [nb, L, HID], BF16, kind="ExternalInput").ap()
    WDT = F8 if fp8 else BF16
    uvw_d = nc.dram_tensor("uvw", [HID, 2 * E + S], WDT, kind="ExternalInput").ap()
    ow_d = nc.dram_tensor("ow", [E, HID], WDT, kind="ExternalInput").ap()
    bias_d = nc.dram_tensor("biasT", [TT, P, L], BF16, kind="ExternalInput").ap()
    cos_d = nc.dram_tensor("cosf", [P, L], F32, kind="ExternalInput").ap()
    sin_d = nc.dram_tensor("sinf", [P, L], F32, kind="ExternalInput").ap()
    prope_d = nc.dram_tensor("prope", [P, P], BF16, kind="ExternalInput").ap()
    qkaff_d = nc.dram_tensor("qkaff", [P, 4], F32, kind="ExternalInput").ap()
    if has_uvb:
        bu_d = nc.dram_tensor("bu", [P, ET + 1], F32, kind="ExternalInput").ap()
        bv_d = nc.dram_tensor("bv", [1, E], BF16, kind="ExternalInput").ap()
    if has_ob:
        ob_d = nc.dram_tensor("ob", [1, HID], BF16, kind="ExternalInput").ap()
    y_d = nc.dram_tensor("y", [nb, L, HID], F32, kind="ExternalOutput").ap()

    with tile.TileContext(nc) as tc, ExitStack() as ctx:
        consts = ctx.enter_context(tc.tile_pool(name="consts", bufs=1))
        xpool = ctx.enter_context(tc.tile_pool(name="xpool", bufs=3))
        xnpool = ctx.enter_context(tc.tile_pool(name="xnpool", bufs=2))
        xntpool = ctx.enter_context(tc.tile_pool(name="xntpool", bufs=2))
        upool = ctx.enter_context(tc.tile_pool(name="upool", bufs=2))
        vpool = ctx.enter_context(tc.tile_pool(name="vpool", bufs=2))
        work = ctx.enter_context(tc.tile_pool(name="work", bufs=2))
        statp = ctx.enter_context(tc.tile_pool(name="statp", bufs=2))
        scp = ctx.enter_context(tc.tile_pool(name="scp", bufs=2))
        gp = ctx.enter_context(tc.tile_pool(name="gp", bufs=2))
        yp = ctx.enter_context(tc.tile_pool(name="yp", bufs=4))

        ps_t = ctx.enter_context(tc.tile_pool(name="ps_t", bufs=2, space="PSUM"))
        ps_mm = ctx.enter_context(tc.tile_pool(name="ps_mm", bufs=4, space="PSUM"))
        ps_y = ctx.enter_context(tc.tile_pool(name="ps_y", bufs=2, space="PSUM"))

        # ---- small constants first (unblock front(0) on DVE/ACT quickly) ----
        epst = consts.tile([P, 1], F32)
        nc.vector.memset(epst[:], EPS)
        magic = consts.tile([P, TT], U32)
        nc.vector.memset(magic[:], RSQRT_MAGIC)
        ident = consts.tile([P, P], BF16)
        make_identity(nc, ident[:])
        # HAM warm-up: PE sits idle at startup waiting for x/weights; keep it
        # busy with throwaway matmuls so the clock gate is at 8/8 (2.4GHz)
        # when the real stream starts.
        wps = ps_t.tile([P, P], F32, tag="pt4")
        for _ in range(85):
            nc.tensor.matmul(wps[:], ident[:], ident[:], start=True, stop=True)
        # prime engines during the initial x DMA: ACT silu table (the only
        # table used) and DVE's internal state so batch-0 layernorm is fast
        prm = consts.tile([P, 6], F32)
        nc.vector.bn_stats(out=prm[:], in_=epst[:])
        prs = consts.tile([P, 1], F32)
        nc.scalar.activation(out=prs[:], in_=epst[:],
                             func=mybir.ActivationFunctionType.Silu)
        qkaff = consts.tile([P, 4], F32)
        nc.sync.dma_start(qkaff[:], qkaff_d)
        prope = consts.tile([P, P], BF16)
        nc.sync.dma_start(prope[:], prope_d)
        if has_uvb:
            bu = consts.tile([P, ET + 1], F32)
            nc.sync.dma_start(bu[:], bu_d)
            bv = consts.tile([1, E], BF16)
            nc.sync.dma_start(bv[:], bv_d)
        if has_ob:
            ob = consts.tile([1, HID], BF16)
            nc.sync.dma_start(ob[:], ob_d)
        if has_uvb or has_ob:
            ones1 = consts.tile([1, P], BF16)
            nc.vector.memset(ones1[:], 1.0)

        g0 = qkaff[:, 0:1]
        b0 = qkaff[:, 1:2]
        g1 = qkaff[:, 2:3]
        b1 = qkaff[:, 3:4]
        SILU = mybir.ActivationFunctionType.Silu
        MUL = mybir.AluOpType.mult
        ADD = mybir.AluOpType.add
        SUB = mybir.AluOpType.subtract
        MAX = mybir.AluOpType.max

        # ---- per-batch stage emitters; state passed via dicts ----
        st = [dict() for _ in range(nb)]

        def front_dma(b):
            """DMA x[b] (bf16 token-major). Emitted well ahead of use."""
            d = st[b]
            x_tok = xpool.tile([P, TT, HID], BF16, tag="x_tok", name=f"x_tok{b}")
            x_r = x_d[b].rearrange("(t p) h -> p t h", p=P)
            for t in range(TT):
                nc.sync.dma_start(x_tok[:, t, :], x_r[:, t, :])
            d["x_tok"] = x_tok

        def front_ln(b):
            """LayerNorm stats + normalize (token-major)."""
            d = st[b]
            x_tok = d["x_tok"]
            xn = xnpool.tile([P, TT, HID], BF16, tag="xn", name=f"xn{b}")
            mvs = statp.tile([P, TT, 2], F32, tag="mvs", name=f"mvs{b}")
            for t in range(TT):
                xin = x_tok[:, t, :].rearrange("p (s c) -> p s c", c=256)
                stats = statp.tile([P, 3, 6], F32, tag="stats")
                for sgi in range(3):
                    nc.vector.bn_stats(out=stats[:, sgi, :], in_=xin[:, sgi, :])
                nc.vector.bn_aggr(out=mvs[:, t, :], in_=stats[:])
            # rstd = 1/sqrt(var+eps) on DVE: fast-inverse-sqrt seed + 2 Newton
            # iterations (avoids the ACT Sqrt table load every batch)
            vpe = statp.tile([P, TT], F32, tag="vpe", name=f"vpe{b}")
            nc.vector.tensor_scalar(out=vpe[:], in0=mvs[:, :, 1], scalar1=EPS,
                                    scalar2=None, op0=ADD)
            sh = statp.tile([P, TT], U32, tag="sh")
            nc.vector.tensor_scalar(out=sh[:], in0=vpe[:].bitcast(U32), scalar1=1,
                                    scalar2=None,
                                    op0=mybir.AluOpType.logical_shift_right)
            yr = statp.tile([P, TT], F32, tag="yr", name=f"yr{b}")
            nc.vector.tensor_tensor(yr[:].bitcast(U32), magic[:], sh[:], SUB)
            t1 = statp.tile([P, TT], F32, tag="t1")
            for _ in range(2):
                nc.vector.tensor_tensor(t1[:], yr[:], yr[:], MUL)
                nc.vector.tensor_tensor(t1[:], t1[:], vpe[:], MUL)
                nc.vector.tensor_scalar(out=t1[:], in0=t1[:], scalar1=-0.5,
                                        scalar2=1.5, op0=MUL, op1=ADD)
                nc.vector.tensor_tensor(yr[:], yr[:], t1[:], MUL)
            for t in range(TT):
                nc.vector.tensor_scalar(
                    out=xn[:, t, :], in0=x_tok[:, t, :],
                    scalar1=mvs[:, t, 0:1], scalar2=yr[:, t:t + 1],
                    op0=SUB, op1=MUL,
                )
            d["xn"] = xn

        def transposes(b):
            d = st[b]
            xn = d["xn"]
            xnT = xntpool.tile([P, KT, L], WDT, tag="xnT", name=f"xnT{b}")
            if b == 0:
                # batch 0: group by token tile so transposes start as soon as
                # the first LN tile lands (startup critical path)
                for t in range(TT):
                    for ks in range(0, KT, 3):
                        pt3 = ps_t.tile([P, 3, P], BF16, tag="pt4")
                        for k in range(ks, ks + 3):
                            nc.tensor.transpose(
                                pt3[:, k - ks, :], xn[:, t, k * P:(k + 1) * P], ident[:])
                        nc.vector.tensor_copy(
                            out=xnT[:, ks:ks + 3, t * P:(t + 1) * P], in_=pt3[:])
            else:
                for k in range(KT):
                    pt4 = ps_t.tile([P, TT, P], BF16, tag="pt4")
                    for t in range(TT):
                        nc.tensor.transpose(pt4[:, t, :], xn[:, t, k * P:(k + 1) * P], ident[:])
                    nc.vector.tensor_copy(
                        out=xnT[:, k, :], in_=pt4.rearrange("p t q -> p (t q)"))
            d["xnT"] = xnT

        def base_group(b):
            d = st[b]
            xnT = d["xnT"]
            ps_b = ps_mm.tile([P, L], F32, tag="ps_mm")
            if fp8:
                for k in range(0, KT, 2):
                    nc.tensor.matmul(
                        ps_b[:], uvw[:, k:k + 2, 2 * E: 2 * E + S], xnT[:, k:k + 2, :],
                        start=(k == 0), stop=(k == KT - 2),
                        perf_mode=mybir.MatmulPerfMode.DoubleRow,
                    )
            else:
                for k in range(KT):
                    nc.tensor.matmul(
                        ps_b[:], uvw[:, k, 2 * E: 2 * E + S], xnT[:, k, :],
                        start=(k == 0), stop=(k == KT - 1),
                    )
            with tc.high_priority(offset=600):
                baseT = work.tile([P, L], F32, tag="baseT", name=f"baseT{b}")
                if has_uvb:
                    nc.scalar.activation(out=baseT[:], in_=ps_b[:], func=SILU,
                                         bias=bu[:, ET:ET + 1], scale=1.0)
                else:
                    nc.scalar.activation(out=baseT[:], in_=ps_b[:], func=SILU)
                # q/k affine on DVE right away (rope matmuls come later)
                for which, gg, bb in (("q", g0, b0), ("k", g1, b1)):
                    pre = work.tile([P, L], BF16, tag=f"{which}pre", name=f"{which}pre{b}")
                    nc.vector.tensor_scalar(
                        out=pre[:], in0=baseT[:], scalar1=gg, scalar2=bb,
                        op0=MUL, op1=ADD,
                    )
                    d[f"{which}pre"] = pre

        def u_groups(b, es):
            d = st[b]
            xnT = d["xnT"]
            if "uT" not in d:
                d["uT"] = upool.tile([P, ET, L], BF16, tag="uT", name=f"uT{b}")
            uT = d["uT"]
            for e in es:
                ps_u = ps_mm.tile([P, L], F32, tag="ps_mm")
                if fp8:
                    for k in range(0, KT, 2):
                        nc.tensor.matmul(
                            ps_u[:], uvw[:, k:k + 2, e * P:(e + 1) * P], xnT[:, k:k + 2, :],
                            start=(k == 0), stop=(k == KT - 2),
                            perf_mode=mybir.MatmulPerfMode.DoubleRow,
                        )
                else:
                    for k in range(KT):
                        nc.tensor.matmul(
                            ps_u[:], uvw[:, k, e * P:(e + 1) * P], xnT[:, k, :],
                            start=(k == 0), stop=(k == KT - 1),
                        )
                if has_uvb:
                    nc.scalar.activation(out=uT[:, e, :], in_=ps_u[:], func=SILU,
                                         bias=bu[:, e:e + 1], scale=1.0)
                else:
                    nc.scalar.activation(out=uT[:, e, :], in_=ps_u[:], func=SILU)

        def rope_mms(b):
            """rotate-half matmuls + combines -> qT/kT bf16.  PSUM comes from
            the transpose pool (idle mid-batch) so the main ps_mm ring never
            waits on the DVE-side rope drain."""
            d = st[b]
            for which in ("q", "k"):
                pre = d[f"{which}pre"]
                ps_r = ps_t.tile([P, L], F32, tag="pt4")
                nc.tensor.matmul(ps_r[:], prope[:], pre[:], start=True, stop=True)
                with tc.high_priority(offset=600):
                    rt = work.tile([P, L], F32, tag="ropetmp")
                    nc.vector.tensor_tensor(rt[:], ps_r[:], sinf[:], MUL)
                    ct = work.tile([P, L], F32, tag="ropecos", name=f"rc_{which}{b}")
                    nc.gpsimd.tensor_tensor(ct[:], pre[:], cosf[:], MUL)
                    qt = work.tile([P, L], BF16, tag=f"{which}T", name=f"{which}T{b}")
                    nc.vector.tensor_tensor(qt[:], ct[:], rt[:], ADD)
                d[which] = qt

        def score_groups(b, mts=range(TT)):
            d = st[b]
            if "scoresT" not in d:
                d["scoresT"] = scp.tile([P, TT, L], WDT, tag="scoresT", name=f"scoresT{b}")
            scoresT = d["scoresT"]
            for mt in mts:
                ps_s = ps_mm.tile([P, L], F32, tag="ps_mm")
                nc.tensor.matmul(
                    ps_s[:], d["k"][:, mt * P:(mt + 1) * P], d["q"][:],
                    start=True, stop=True,
                )
                with tc.high_priority(offset=600):
                    stmp = work.tile([P, L], BF16, tag="stmp", bufs=4)
                    nc.vector.tensor_tensor(stmp[:], ps_s[:], biasT[:, mt, :], ADD)
                    # relu^2 in one op: max(x,0) * x
                    nc.vector.scalar_tensor_tensor(
                        out=scoresT[:, mt, :], in0=stmp[:], scalar=0.0,
                        in1=stmp[:], op0=MAX, op1=MUL)

        def v_groups(b, ts=range(TT)):
            d = st[b]
            xnT = d["xnT"]
            if "v_sb" not in d:
                d["v_sb"] = vpool.tile([P, TT, 3, 512], WDT, tag="v_sb", name=f"v_sb{b}")
            v_sb = d["v_sb"]
            for t in ts:
                for c in range(3):
                    ps_v = ps_mm.tile([P, 512], F32, tag="ps_mm")
                    if fp8:
                        for k in range(0, KT, 2):
                            nc.tensor.matmul(
                                ps_v[:], xnT[:, k:k + 2, t * P:(t + 1) * P],
                                uvw[:, k:k + 2, E + c * 512: E + (c + 1) * 512],
                                start=(k == 0), stop=(k == KT - 2 and not has_uvb),
                                perf_mode=mybir.MatmulPerfMode.DoubleRow,
                            )
                    else:
                        for k in range(KT):
                            nc.tensor.matmul(
                                ps_v[:], xnT[:, k, t * P:(t + 1) * P],
                                uvw[:, k, E + c * 512: E + (c + 1) * 512],
                                start=(k == 0), stop=(k == KT - 1 and not has_uvb),
                            )
                    if has_uvb:
                        nc.tensor.matmul(
                            ps_v[:], ones1[:],
                            bv[:, c * 512:(c + 1) * 512],
                            start=False, stop=True, skip_group_check=True,
                        )
                    nc.scalar.activation(out=v_sb[:, t, c, :], in_=ps_v[:], func=SILU)

        def attnv_groups(b):
            d = st[b]
            gatedT = gp.tile([P, ET, L], WDT, tag="gatedT", name=f"gatedT{b}")
            for e in range(ET):
                c, el = divmod(e, 4)
                ps_a = ps_mm.tile([P, L], F32, tag="ps_mm")
                if fp8:
                    for mt in range(0, TT, 2):
                        nc.tensor.matmul(
                            ps_a[:], d["v_sb"][:, mt:mt + 2, c, el * P:(el + 1) * P],
                            d["scoresT"][:, mt:mt + 2, :],
                            start=(mt == 0), stop=(mt == TT - 2),
                            perf_mode=mybir.MatmulPerfMode.DoubleRow,
                        )
                else:
                    for mt in range(TT):
                        nc.tensor.matmul(
                            ps_a[:], d["v_sb"][:, mt, c, el * P:(el + 1) * P],
                            d["scoresT"][:, mt, :],
                            start=(mt == 0), stop=(mt == TT - 1),
                        )
                nc.vector.tensor_tensor(
                    gatedT[:, e, :], ps_a[:], d["uT"][:, e, :], MUL)
            d["gatedT"] = gatedT

        def oproj(b):
            d = st[b]
            last = b == nb - 1
            for t in range(TT):
                y_tok = yp.tile([P, HID], F32, tag="y_tok")
                for c in range(2):
                    ps_o = ps_y.tile([P, HID // 2], F32, tag="ps_y")
                    if fp8:
                        for e in range(0, ET, 2):
                            nc.tensor.matmul(
                                ps_o[:], d["gatedT"][:, e:e + 2, t * P:(t + 1) * P],
                                ow[:, e:e + 2, c * (HID // 2):(c + 1) * (HID // 2)],
                                start=(e == 0), stop=(e == ET - 2 and not has_ob),
                                perf_mode=mybir.MatmulPerfMode.DoubleRow,
                            )
                    else:
                        for e in range(ET):
                            nc.tensor.matmul(
                                ps_o[:], d["gatedT"][:, e, t * P:(t + 1) * P],
                                ow[:, e, c * (HID // 2):(c + 1) * (HID // 2)],
                                start=(e == 0), stop=(e == ET - 1 and not has_ob),
                            )
                    if has_ob:
                        nc.tensor.matmul(
                            ps_o[:], ones1[:],
                            ob[:, c * (HID // 2):(c + 1) * (HID // 2)],
                            start=False, stop=True, skip_group_check=True,
                        )
                    scale = 1.0 / LAM if fp8 else 1.0
                    hs = slice(c * (HID // 2), (c + 1) * (HID // 2))
                    if last:
                        # nothing follows: fused drain on the idle DVE
                        # shortens the pipeline tail
                        nc.vector.scalar_tensor_tensor(
                            out=y_tok[:, hs], in0=ps_o[:], scalar=scale,
                            in1=d["x_tok"][:, t, hs], op0=MUL, op1=ADD)
                    else:
                        # DVE is busy with next-batch LN; ACT (Copy is on the
                        # silu table: no table load) + gpsimd add instead
                        y1 = work.tile([P, HID // 2], F32, tag="y1")
                        nc.scalar.activation(
                            out=y1[:], in_=ps_o[:],
                            func=mybir.ActivationFunctionType.Copy,
                            scale=scale,
                        )
                        nc.gpsimd.tensor_tensor(
                            y_tok[:, hs], y1[:], d["x_tok"][:, t, hs], ADD)
                    nc.sync.dma_start(y_d[b, t * P:(t + 1) * P, hs], y_tok[:, hs])
            st[b] = {}

        # ---- software-pipelined emission ----
        # front_dma(0) first so x[0] DMA starts immediately; weight DMAs are
        # column-grouped in first-use order right behind it.
        front_dma(0)
        uvw = consts.tile([P, KT, 2 * E + S], WDT)
        uvw_r = uvw_d.rearrange("(k p) f -> p k f", p=P)
        # base cols + first two u tiles, then u2-5, u6-11, then v halves
        nc.sync.dma_start(uvw[:, :, 2 * E:2 * E + S], uvw_r[:, :, 2 * E:2 * E + S])
        nc.sync.dma_start(uvw[:, :, 0:2 * P], uvw_r[:, :, 0:2 * P])
        nc.sync.dma_start(uvw[:, :, 2 * P:6 * P], uvw_r[:, :, 2 * P:6 * P])
        cosf = consts.tile([P, L], F32)
        nc.sync.dma_start(cosf[:], cos_d)
        sinf = consts.tile([P, L], F32)
        nc.sync.dma_start(sinf[:], sin_d)
        nc.sync.dma_start(uvw[:, :, 6 * P:ET * P], uvw_r[:, :, 6 * P:ET * P])
        biasT = consts.tile([P, TT, L], BF16)
        nc.sync.dma_start(biasT[:], bias_d.rearrange("t p n -> p t n"))
        nc.sync.dma_start(uvw[:, :, E:E + 768], uvw_r[:, :, E:E + 768])
        nc.sync.dma_start(uvw[:, :, E + 768:2 * E], uvw_r[:, :, E + 768:2 * E])
        ow = consts.tile([P, ET, HID], WDT)
        ow_r = ow_d.rearrange("(k p) f -> p k f", p=P)
        nc.sync.dma_start(ow[:], ow_r[:])

        front_ln(0)
        for b in range(nb):
            transposes(b)
            base_group(b)
            u_groups(b, range(0, 6))
            rope_mms(b)
            u_groups(b, range(6, ET))
            score_groups(b)
            if b + 1 < nb:
                front_dma(b + 1)
            v_groups(b)
            attnv_groups(b)
            if b + 1 < nb:
                front_ln(b + 1)
            oproj(b)

    nc.compile()
    return nc


def _host_prep(x, ln_gamma, ln_beta, uv_W, uv_b, gamma_qk, beta_qk, w_rel, o_W, o_b,
               fp8=False):
    """Host-side input preprocessing: fold LN affine into uv_W, 1/L into the q
    affine, expand the Toeplitz bias, build rope tables.

    fp8 mode: weights are cast to float8_e4m3 and the attention scores are
    scaled by LAM (sqrt(LAM) folded into the q affine and the Toeplitz bias;
    relu^2 turns that into LAM; the o-projection PSUM drain divides it out).
    fp8's min normal is 2^-6 -- unscaled relu^2 scores (~1e-4) would land in
    subnormals and quantize to garbage."""
    f32 = np.float32
    sq = f32(np.sqrt(LAM)) if fp8 else f32(1.0)
    uv_W = np.asarray(uv_W, f32)
    uv_b_eff = (np.asarray(ln_beta, f32) @ uv_W + np.asarray(uv_b, f32)).astype(f32)
    uv_W_eff = (np.asarray(ln_gamma, f32)[:, None] * uv_W).astype(f32)

    gamma_qk = np.asarray(gamma_qk, f32)
    beta_qk = np.asarray(beta_qk, f32)
    qkaff = np.stack(
        [gamma_qk[0] * sq / f32(L), beta_qk[0] * sq / f32(L), gamma_qk[1], beta_qk[1]],
        axis=1,
    ).astype(f32)                                           # [128, 4]

    # rope tables, feature-major: cos/sin[s, n] = cos/sin(n * invf[s % 64])
    inv_freq = np.power(f32(10000.0), -np.arange(HALF, dtype=f32) / f32(HALF))
    sinusoid = np.arange(L, dtype=f32)[None, :] * inv_freq[:, None]   # [64, 512]
    cosf = np.concatenate([np.cos(sinusoid), np.cos(sinusoid)], 0).astype(f32)
    sinf = np.concatenate([np.sin(sinusoid), np.sin(sinusoid)], 0).astype(f32)

    # signed rotate-half permutation, as lhsT: out[m,n] = sum_s lhsT[s,m] in[s,n]
    prope = np.zeros((S, S), f32)
    for m in range(HALF):
        prope[m + HALF, m] = -1.0
    for m in range(HALF, S):
        prope[m - HALF, m] = 1.0

    # Toeplitz bias, transposed orientation: biasT[mt, p, n] = w_rel[128*mt+p-n+511]
    w_rel = np.asarray(w_rel, f32)
    idx = (np.arange(L)[:, None] - np.arange(L)[None, :] + (L - 1))   # [m, n]
    biasT = (w_rel[idx].reshape(TT, P, L) * sq).astype(ml_dtypes.bfloat16)

    bu = np.stack(
        [uv_b_eff[e * P:(e + 1) * P] for e in range(ET)] + [uv_b_eff[2 * E: 2 * E + S]],
        axis=1,
    ).astype(f32)                                           # [128, 13]

    has_uvb = bool(np.any(uv_b_eff != 0))
    o_b = np.asarray(o_b, f32)
    has_ob = bool(np.any(o_b != 0))

    wnp = mybir.dt.np(F8) if fp8 else ml_dtypes.bfloat16
    shared = {
        "uvw": uv_W_eff.astype(wnp),
        "ow": np.asarray(o_W, f32).astype(wnp),
        "biasT": biasT,
        "cosf": cosf,
        "sinf": sinf,
        "prope": prope.astype(ml_dtypes.bfloat16),
        "qkaff": qkaff,
    }
    if has_uvb:
        shared["bu"] = bu
        shared["bv"] = uv_b_eff[E:2 * E].reshape(1, E).astype(ml_dtypes.bfloat16)
    if has_ob:
        shared["ob"] = o_b.reshape(1, HID).astype(ml_dtypes.bfloat16)
    return shared, has_uvb, has_ob


_prog_cache = {}


def run(inputs, trace=False, trace_kwargs=None, fp8=USE_FP8):
    x = np.asarray(inputs["x"], np.float32).astype(ml_dtypes.bfloat16)
    shared, has_uvb, has_ob = _host_prep(**inputs, fp8=fp8)
    key = (has_uvb, has_ob, fp8)
    if key not in _prog_cache:
        _prog_cache[key] = _build_program(has_uvb, has_ob, fp8=fp8)
    nc = _prog_cache[key]
    in_maps = [
        {"x": np.ascontiguousarray(x[i * NB:(i + 1) * NB]), **shared}
        for i in range(N_CORES)
    ]
    kw = {}
    if trace:
        kw = dict(trace=True, trace_kwargs=trace_kwargs or {})
    try:
        res = bass_utils.run_bass_kernel_spmd(nc, in_maps, core_ids=list(range(N_CORES)), **kw)
    except Exception:
        import time as _time
        _time.sleep(10)
        res = bass_utils.run_bass_kernel_spmd(nc, in_maps, core_ids=list(range(N_CORES)), **kw)
    y = np.concatenate([res.results[i]["y"] for i in range(N_CORES)], axis=0)
    return y, res


def kernel(**inputs) -> np.ndarray:
    y, _ = run(inputs, trace=False)
    return y
